# revision 2
# baseline (speedup 1.0000x reference)
"""Trainium2 Bass kernel for causal multi-head attention (B=2, S=2048, D=2048, H=16).

Sharding: DP over batch (2) x TP over heads (4 groups of 4 heads) = 8 cores.
Each core computes, for its batch b and head group g:
  - Q/K/V projections restricted to its 512 head-dims (transposed layouts)
  - causal attention for its 4 heads (scores produced directly transposed,
    softmax sums via ones-matmul, normalization folded in post-PV)
  - partial output projection through its 512 columns of wo
Host sums the 4 TP partials per batch (the "all-reduce" of the hint, done in
numpy on the gathered partials) and stacks the 2 batches.

All matmuls run in bf16 (fp32 PSUM accumulation); host pre-converts inputs.
"""

import numpy as np
import ml_dtypes

import concourse.bacc as bacc
import concourse.tile as tile
from concourse import mybir
from concourse.bass_utils import run_bass_kernel_spmd

BF16 = ml_dtypes.bfloat16

# Full problem sizes (hardcoded; grading calls kernel() with these shapes).
B, S, D, H = 2, 2048, 2048, 16
HD = 128          # head dim
P = 128           # SBUF partitions
CH = 512          # matmul moving-dim chunk
N_CORES = 8
M_CORE = D // 4   # head-dims per core (4 heads x 128)


def build_core_kernel(nc, io, S=S, D=D, M_CORE=M_CORE):
    """Emit the per-core Tile program. io maps names -> bass APs."""
    E_T = D // P        # e (contraction) tiles
    S_T = S // P        # s tiles
    S_C = S // CH       # s chunks
    H_C = M_CORE // P   # heads on this core
    D_C = D // CH       # output e chunks
    SUB = CH // P       # 128-tiles per chunk
    bf = mybir.dt.bfloat16
    f32 = mybir.dt.float32
    SCALE = 1.0 / float(np.sqrt(np.float32(HD)))

    qT, kT, vT = io["qT"], io["kT"], io["vT"]
    wqT, wkT, wvT, woT = io["wqT"], io["wkT"], io["wvT"], io["woT"]
    cmask = io["cmask"]
    out = io["out"]

    with tile.TileContext(nc) as tc:
        import contextlib

        with contextlib.ExitStack() as ctx:
            # ---- long-lived pools ----
            wpool = ctx.enter_context(tc.tile_pool(name="wpool", bufs=2))
            wop = ctx.enter_context(tc.tile_pool(name="wop", bufs=1))
            cons = ctx.enter_context(tc.tile_pool(name="cons", bufs=1))
            projo = ctx.enter_context(tc.tile_pool(name="projo", bufs=1))
            xin = ctx.enter_context(tc.tile_pool(name="xin", bufs=8))

            # constants
            mask_sb = cons.tile([P, SUB, CH], bf, name="mask_sb")
            nc.sync.dma_start(out=mask_sb, in_=cmask.rearrange("p (s c) -> p s c", s=SUB))
            ones_col = cons.tile([P, 1], bf, name="ones_col")
            nc.vector.memset(ones_col, 1.0)
            ones_row = cons.tile([1, P], bf, name="ones_row")
            nc.vector.memset(ones_row, 1.0)

            # wo weights resident
            woT_sb = wop.tile([P, H_C, D], bf, name="woT_sb")
            for h in range(H_C):
                nc.sync.dma_start(out=woT_sb[:, h, :], in_=woT[h * P:(h + 1) * P, :])

            # projection outputs
            xqT_sb = projo.tile([P, H_C, S], bf, name="xqT_sb")
            xkT_sb = projo.tile([P, H_C, S], bf, name="xkT_sb")
            xv_sb = projo.tile([P, S_T, M_CORE], bf, name="xv_sb")

            # ---- stage 1: projections ----
            with tc.tile_pool(name="ps_proj", bufs=6, space="PSUM") as psp:
                for name, w_dram, x_dram in (("q", wqT, qT), ("k", wkT, kT)):
                    w_sb = wpool.tile([P, E_T, M_CORE], bf, tag="wproj", name=f"w{name}_sb")
                    for e_t in range(E_T):
                        nc.sync.dma_start(out=w_sb[:, e_t, :],
                                          in_=w_dram[e_t * P:(e_t + 1) * P, :])
                    dst = xqT_sb if name == "q" else xkT_sb
                    for s_c in range(S_C):
                        ps = [psp.tile([P, CH], f32, tag="proj", name=f"ps_{name}{s_c}_{m}")
                              for m in range(H_C)]
                        for e_t in range(E_T):
                            xt = xin.tile([P, CH], bf, tag="xin", name=f"x{name}{s_c}_{e_t}")
                            nc.sync.dma_start(
                                out=xt,
                                in_=x_dram[e_t * P:(e_t + 1) * P, s_c * CH:(s_c + 1) * CH])
                            for m in range(H_C):
                                nc.tensor.matmul(ps[m], lhsT=w_sb[:, e_t, m * P:(m + 1) * P],
                                                 rhs=xt, start=(e_t == 0), stop=(e_t == E_T - 1))
                        for m in range(H_C):
                            nc.vector.tensor_copy(dst[:, m, s_c * CH:(s_c + 1) * CH], ps[m])

                # v projection: xv[s, m] (non-transposed), input blocks stationary
                wv_sb = wpool.tile([P, E_T, M_CORE], bf, tag="wproj", name="wv_sb")
                for e_t in range(E_T):
                    nc.sync.dma_start(out=wv_sb[:, e_t, :],
                                      in_=wvT[e_t * P:(e_t + 1) * P, :])
                for s_c in range(S_C):
                    ps = [psp.tile([P, M_CORE], f32, tag="proj", name=f"ps_v{s_c}_{m}")
                          for m in range(SUB)]
                    for e_t in range(E_T):
                        xt = xin.tile([P, CH], bf, tag="xin", name=f"xv{s_c}_{e_t}")
                        nc.sync.dma_start(
                            out=xt,
                            in_=vT[e_t * P:(e_t + 1) * P, s_c * CH:(s_c + 1) * CH])
                        for s_sub in range(SUB):
                            nc.tensor.matmul(ps[s_sub], lhsT=xt[:, s_sub * P:(s_sub + 1) * P],
                                             rhs=wv_sb[:, e_t, :],
                                             start=(e_t == 0), stop=(e_t == E_T - 1))
                    for s_sub in range(SUB):
                        nc.vector.tensor_copy(xv_sb[:, s_c * SUB + s_sub, :], ps[s_sub])

            # ---- stage 2: attention ----
            attnT_sb = projo.tile([P, H_C, S], bf, name="attnT_sb")
            with tc.tile_pool(name="ps_sc", bufs=2, space="PSUM") as pssc, \
                 tc.tile_pool(name="ps_pv", bufs=2, space="PSUM") as pspv, \
                 tc.tile_pool(name="ps_sum", bufs=2, space="PSUM") as pssum, \
                 tc.tile_pool(name="ps_bc", bufs=1, space="PSUM") as psbc, \
                 tc.tile_pool(name="sm", bufs=2) as smp, \
                 tc.tile_pool(name="expt", bufs=2) as expp:
                for h in range(H_C):
                    for c in range(S_C):
                        n_t = SUB * (c + 1)
                        exp_t = expp.tile([P, S_T, CH], bf, tag="expT", name=f"exp_{h}_{c}")
                        for t in range(n_t):
                            ps_sc = pssc.tile([P, CH], f32, tag="sc", name=f"sc_{h}_{c}_{t}")
                            nc.tensor.matmul(ps_sc,
                                             lhsT=xkT_sb[:, h, t * P:(t + 1) * P],
                                             rhs=xqT_sb[:, h, c * CH:(c + 1) * CH],
                                             start=True, stop=True)
                            nc.scalar.activation(exp_t[:, t, :], ps_sc,
                                                 mybir.ActivationFunctionType.Exp,
                                                 scale=SCALE)
                            p = t - SUB * c
                            if p >= 0:
                                nc.vector.tensor_mul(exp_t[:, t, :], exp_t[:, t, :],
                                                     mask_sb[:, p, :])
                        ps_pv = pspv.tile([P, CH], f32, tag="pv", name=f"pv_{h}_{c}")
                        for t in range(n_t):
                            nc.tensor.matmul(ps_pv,
                                             lhsT=xv_sb[:, t, h * P:(h + 1) * P],
                                             rhs=exp_t[:, t, :],
                                             start=(t == 0), stop=(t == n_t - 1))
                        ps_sum = pssum.tile([1, CH], f32, tag="sums", name=f"sum_{h}_{c}")
                        for t in range(n_t):
                            nc.tensor.matmul(ps_sum, lhsT=ones_col, rhs=exp_t[:, t, :],
                                             start=(t == 0), stop=(t == n_t - 1))
                        recip = smp.tile([1, CH], bf, tag="recip", name=f"rc_{h}_{c}")
                        with nc.allow_low_precision(reason="softmax recip in bf16"):
                            nc.vector.reciprocal(recip, ps_sum)
                        ps_bc = psbc.tile([P, CH], f32, tag="bc", name=f"bc_{h}_{c}")
                        nc.tensor.matmul(ps_bc, lhsT=ones_row, rhs=recip,
                                         start=True, stop=True)
                        bc_sb = smp.tile([P, CH], f32, tag="bcs", name=f"bcs_{h}_{c}")
                        nc.vector.tensor_copy(bc_sb, ps_bc)
                        nc.vector.tensor_mul(attnT_sb[:, h, c * CH:(c + 1) * CH],
                                             ps_pv, bc_sb)

            # ---- stage 3: output projection (partial) ----
            with tc.tile_pool(name="ps_wo", bufs=6, space="PSUM") as pswo, \
                 tc.tile_pool(name="outp", bufs=3) as outp:
                for s_t in range(S_T):
                    out_sb = outp.tile([P, D], f32, tag="out", name=f"out_{s_t}")
                    ps = [pswo.tile([P, CH], f32, tag="wops", name=f"wo_{s_t}_{e}")
                          for e in range(D_C)]
                    for h in range(H_C):
                        for e_c in range(D_C):
                            nc.tensor.matmul(ps[e_c],
                                             lhsT=attnT_sb[:, h, s_t * P:(s_t + 1) * P],
                                             rhs=woT_sb[:, h, e_c * CH:(e_c + 1) * CH],
                                             start=(h == 0), stop=(h == H_C - 1))
                    for e_c in range(D_C):
                        nc.vector.tensor_copy(out_sb[:, e_c * CH:(e_c + 1) * CH], ps[e_c])
                    nc.sync.dma_start(out=out[s_t * P:(s_t + 1) * P, :], in_=out_sb)


def build_nc(S=S, D=D, M_CORE=M_CORE):
    nc = bacc.Bacc("TRN2", target_bir_lowering=False, debug=False, num_devices=N_CORES)
    bf = mybir.dt.bfloat16
    io = {
        "qT": nc.dram_tensor("qT", [D, S], bf, kind="ExternalInput").ap(),
        "kT": nc.dram_tensor("kT", [D, S], bf, kind="ExternalInput").ap(),
        "vT": nc.dram_tensor("vT", [D, S], bf, kind="ExternalInput").ap(),
        "wqT": nc.dram_tensor("wqT", [D, M_CORE], bf, kind="ExternalInput").ap(),
        "wkT": nc.dram_tensor("wkT", [D, M_CORE], bf, kind="ExternalInput").ap(),
        "wvT": nc.dram_tensor("wvT", [D, M_CORE], bf, kind="ExternalInput").ap(),
        "woT": nc.dram_tensor("woT", [M_CORE, D], bf, kind="ExternalInput").ap(),
        "cmask": nc.dram_tensor("cmask", [P, (CH // P) * CH], bf,
                                kind="ExternalInput").ap(),
        "out": nc.dram_tensor("out", [S, D], mybir.dt.float32,
                              kind="ExternalOutput").ap(),
    }
    build_core_kernel(nc, io, S=S, D=D, M_CORE=M_CORE)
    nc.compile()
    return nc


def make_mask():
    i = np.arange(P)[:, None]
    j = np.arange(CH)[None, :]
    m = np.concatenate(
        [(j >= P * p + i).astype(np.float32) for p in range(CH // P)], axis=1)
    return m.astype(BF16)


def prep_in_maps(q, k, v, wq, wk, wv, wo):
    cmask = make_mask()
    qT = [np.ascontiguousarray(q[b].T).astype(BF16) for b in range(B)]
    kT = [np.ascontiguousarray(k[b].T).astype(BF16) for b in range(B)]
    vT = [np.ascontiguousarray(v[b].T).astype(BF16) for b in range(B)]
    in_maps = []
    for c in range(N_CORES):
        b, g = divmod(c, N_CORES // B)
        M = slice(g * M_CORE, (g + 1) * M_CORE)
        in_maps.append({
            "qT": qT[b], "kT": kT[b], "vT": vT[b],
            "wqT": np.ascontiguousarray(wq[M, :].T).astype(BF16),
            "wkT": np.ascontiguousarray(wk[M, :].T).astype(BF16),
            "wvT": np.ascontiguousarray(wv[M, :].T).astype(BF16),
            "woT": np.ascontiguousarray(wo[:, M].T).astype(BF16),
            "cmask": cmask,
        })
    return in_maps


def run(inputs, trace=False):
    nc = build_nc()
    in_maps = prep_in_maps(inputs["q"], inputs["k"], inputs["v"],
                           inputs["wq"], inputs["wk"], inputs["wv"], inputs["wo"])
    res = run_bass_kernel_spmd(nc, in_maps, core_ids=list(range(N_CORES)),
                               trace=trace)
    g = N_CORES // B
    out = np.stack([
        np.sum([res.results[b * g + i]["out"] for i in range(g)], axis=0)
        for b in range(B)
    ]).astype(np.float32)
    return out, res


def kernel(**inputs):
    out, _ = run(inputs, trace=False)
    return out


# revision 3
# speedup vs baseline: 1.0042x; 1.0042x over previous
"""Trainium2 Bass kernel for causal multi-head attention (B=2, S=2048, D=2048, H=16).

Sharding: DP over batch (2) x TP over heads (4 groups of 4 heads) = 8 cores.
Each core computes, for its batch b and head group g:
  - Q/K/V projections restricted to its 512 head-dims (transposed layouts)
  - causal attention for its 4 heads (scores produced directly transposed,
    softmax sums via ones-matmul, normalization folded in post-PV)
  - partial output projection through its 512 columns of wo
Host sums the 4 TP partials per batch (the "all-reduce" of the hint, done in
numpy on the gathered partials) and stacks the 2 batches.

All matmuls run in bf16 (fp32 PSUM accumulation); host pre-converts inputs.
"""

import numpy as np
import ml_dtypes

import concourse.bacc as bacc
import concourse.tile as tile
from concourse import mybir
from concourse.bass_utils import run_bass_kernel_spmd

BF16 = ml_dtypes.bfloat16

# Full problem sizes (hardcoded; grading calls kernel() with these shapes).
B, S, D, H = 2, 2048, 2048, 16
HD = 128          # head dim
P = 128           # SBUF partitions
CH = 512          # matmul moving-dim chunk
N_CORES = 8
M_CORE = D // 4   # head-dims per core (4 heads x 128)


def build_core_kernel(nc, io, S=S, D=D, M_CORE=M_CORE):
    """Emit the per-core Tile program (single causal-chunk pipeline).

    For each 512-wide sequence chunk c: project q/k/v for chunk c, run
    attention for all heads on queries of chunk c (keys/values from chunks
    <= c, which are already projected), then the wo partial for chunk c's
    rows. This keeps TensorE dense (projection matmuls of chunk c+1 fill the
    ACT-bound stretches of attention on chunk c).
    """
    E_T = D // P        # e (contraction) tiles
    S_T = S // P        # s tiles
    S_C = S // CH       # s chunks
    H_C = M_CORE // P   # heads on this core
    D_C = D // CH       # output e chunks
    SUB = CH // P       # 128-tiles per chunk
    bf = mybir.dt.bfloat16
    f32 = mybir.dt.float32
    SCALE = 1.0 / float(np.sqrt(np.float32(HD)))

    qT, kT, vT = io["qT"], io["kT"], io["vT"]
    wqT, wkT, wvT, woT = io["wqT"], io["wkT"], io["wvT"], io["woT"]
    cmask = io["cmask"]
    out = io["out"]

    import contextlib

    with tile.TileContext(nc) as tc, contextlib.ExitStack() as ctx:
        wpool = ctx.enter_context(tc.tile_pool(name="wpool", bufs=1))
        cons = ctx.enter_context(tc.tile_pool(name="cons", bufs=1))
        projo = ctx.enter_context(tc.tile_pool(name="projo", bufs=1))
        xin = ctx.enter_context(tc.tile_pool(name="xin", bufs=22))
        expp = ctx.enter_context(tc.tile_pool(name="expt", bufs=1))
        smp = ctx.enter_context(tc.tile_pool(name="sm", bufs=2))
        outp = ctx.enter_context(tc.tile_pool(name="outp", bufs=2))
        acc = ctx.enter_context(tc.tile_pool(name="ps_acc", bufs=3, space="PSUM"))
        pssc = ctx.enter_context(tc.tile_pool(name="ps_sc", bufs=3, space="PSUM"))
        pspv = ctx.enter_context(tc.tile_pool(name="ps_pv", bufs=1, space="PSUM"))
        psbc = ctx.enter_context(tc.tile_pool(name="ps_bc", bufs=1, space="PSUM"))

        # projection weights resident (per-e-tile DMAs so the first matmuls
        # only wait on their own slice)
        w_sbs = {}
        for name, w_dram in (("q", wqT), ("k", wkT), ("v", wvT)):
            w_sb = wpool.tile([P, E_T, M_CORE], bf, name=f"w{name}_sb")
            for e_t in range(E_T):
                nc.sync.dma_start(out=w_sb[:, e_t, :],
                                  in_=w_dram[e_t * P:(e_t + 1) * P, :])
            w_sbs[name] = w_sb

        mask_sb = cons.tile([P, SUB, CH], bf, name="mask_sb")
        nc.sync.dma_start(out=mask_sb, in_=cmask.rearrange("p (s c) -> p s c", s=SUB))
        ones_mat = cons.tile([P, P], bf, name="ones_mat")
        nc.vector.memset(ones_mat, 1.0)
        woT_sb = cons.tile([P, H_C, D], bf, name="woT_sb")

        xqT_sb = projo.tile([P, H_C, S], bf, name="xqT_sb")
        xkT_sb = projo.tile([P, H_C, S], bf, name="xkT_sb")
        xv_sb = projo.tile([P, S_T, M_CORE], bf, name="xv_sb")
        attnT_sb = projo.tile([P, H_C, S], bf, name="attnT_sb")

        for c in range(S_C):
            csl = slice(c * CH, (c + 1) * CH)
            # ---- projections for chunk c ----
            for name, x_dram in (("q", qT), ("k", kT), ("v", vT)):
                w_sb = w_sbs[name]
                xts = []
                for e_t in range(E_T):
                    xt = xin.tile([P, CH], bf, tag="xin", name=f"x{name}{c}_{e_t}")
                    nc.sync.dma_start(
                        out=xt, in_=x_dram[e_t * P:(e_t + 1) * P, csl])
                    xts.append(xt)
                if name in ("q", "k"):
                    dst = xqT_sb if name == "q" else xkT_sb
                    for m in range(H_C):
                        ps = acc.tile([P, CH], f32, tag="acc", name=f"ps_{name}{c}_{m}")
                        for e_t in range(E_T):
                            nc.tensor.matmul(ps, lhsT=w_sb[:, e_t, m * P:(m + 1) * P],
                                             rhs=xts[e_t],
                                             start=(e_t == 0), stop=(e_t == E_T - 1))
                        nc.vector.tensor_copy(dst[:, m, csl], ps)
                else:
                    for s_sub in range(SUB):
                        ps = acc.tile([P, M_CORE], f32, tag="acc", name=f"ps_v{c}_{s_sub}")
                        for e_t in range(E_T):
                            nc.tensor.matmul(ps,
                                             lhsT=xts[e_t][:, s_sub * P:(s_sub + 1) * P],
                                             rhs=w_sb[:, e_t, :],
                                             start=(e_t == 0), stop=(e_t == E_T - 1))
                        nc.vector.tensor_copy(xv_sb[:, c * SUB + s_sub, :], ps)

            if c == 0:
                # wo weights: only needed from the first wo stage on; DMA here
                # so they don't delay the chunk-0 input stream
                for h in range(H_C):
                    nc.sync.dma_start(out=woT_sb[:, h, :],
                                      in_=woT[h * P:(h + 1) * P, :])

            # ---- attention for chunk c, all heads ----
            n_t = SUB * (c + 1)
            for h in range(H_C):
                exp_t = expp.tile([P, S_T, CH], bf, tag="expT", name=f"exp_{h}_{c}")
                for t in range(n_t):
                    ps_sc = pssc.tile([P, CH], f32, tag="sc", name=f"sc_{h}_{c}_{t}")
                    nc.tensor.matmul(ps_sc,
                                     lhsT=xkT_sb[:, h, t * P:(t + 1) * P],
                                     rhs=xqT_sb[:, h, csl],
                                     start=True, stop=True)
                    nc.scalar.activation(exp_t[:, t, :], ps_sc,
                                         mybir.ActivationFunctionType.Exp,
                                         scale=SCALE)
                    p = t - SUB * c
                    if p >= 0:
                        nc.vector.tensor_mul(exp_t[:, t, :], exp_t[:, t, :],
                                             mask_sb[:, p, :])
                ps_pv = pspv.tile([P, CH], f32, tag="pv", name=f"pv_{h}_{c}")
                for t in range(n_t):
                    nc.tensor.matmul(ps_pv,
                                     lhsT=xv_sb[:, t, h * P:(h + 1) * P],
                                     rhs=exp_t[:, t, :],
                                     start=(t == 0), stop=(t == n_t - 1))
                # broadcast column-sums: all-ones stationary -> every psum
                # partition gets sum_sk(exp)
                ps_bc = psbc.tile([P, CH], f32, tag="bc", name=f"bc_{h}_{c}")
                for t in range(n_t):
                    nc.tensor.matmul(ps_bc, lhsT=ones_mat, rhs=exp_t[:, t, :],
                                     start=(t == 0), stop=(t == n_t - 1))
                bc_sb = smp.tile([P, CH], f32, tag="bcs", name=f"bcs_{h}_{c}")
                nc.vector.reciprocal(bc_sb, ps_bc)
                nc.vector.tensor_mul(attnT_sb[:, h, csl], ps_pv, bc_sb)

            # ---- wo partial for chunk c's rows ----
            for s_t in range(c * SUB, (c + 1) * SUB):
                out_sb = outp.tile([P, D], f32, tag="out", name=f"out_{s_t}")
                for e_c in range(D_C):
                    ps = acc.tile([P, CH], f32, tag="acc", name=f"wo_{s_t}_{e_c}")
                    for h in range(H_C):
                        nc.tensor.matmul(ps,
                                         lhsT=attnT_sb[:, h, s_t * P:(s_t + 1) * P],
                                         rhs=woT_sb[:, h, e_c * CH:(e_c + 1) * CH],
                                         start=(h == 0), stop=(h == H_C - 1))
                    nc.vector.tensor_copy(out_sb[:, e_c * CH:(e_c + 1) * CH], ps)
                nc.sync.dma_start(out=out[s_t * P:(s_t + 1) * P, :], in_=out_sb)


def build_nc(S=S, D=D, M_CORE=M_CORE):
    nc = bacc.Bacc("TRN2", target_bir_lowering=False, debug=False, num_devices=N_CORES)
    bf = mybir.dt.bfloat16
    io = {
        "qT": nc.dram_tensor("qT", [D, S], bf, kind="ExternalInput").ap(),
        "kT": nc.dram_tensor("kT", [D, S], bf, kind="ExternalInput").ap(),
        "vT": nc.dram_tensor("vT", [D, S], bf, kind="ExternalInput").ap(),
        "wqT": nc.dram_tensor("wqT", [D, M_CORE], bf, kind="ExternalInput").ap(),
        "wkT": nc.dram_tensor("wkT", [D, M_CORE], bf, kind="ExternalInput").ap(),
        "wvT": nc.dram_tensor("wvT", [D, M_CORE], bf, kind="ExternalInput").ap(),
        "woT": nc.dram_tensor("woT", [M_CORE, D], bf, kind="ExternalInput").ap(),
        "cmask": nc.dram_tensor("cmask", [P, (CH // P) * CH], bf,
                                kind="ExternalInput").ap(),
        "out": nc.dram_tensor("out", [S, D], mybir.dt.float32,
                              kind="ExternalOutput").ap(),
    }
    build_core_kernel(nc, io, S=S, D=D, M_CORE=M_CORE)
    nc.compile()
    return nc


def make_mask():
    i = np.arange(P)[:, None]
    j = np.arange(CH)[None, :]
    m = np.concatenate(
        [(j >= P * p + i).astype(np.float32) for p in range(CH // P)], axis=1)
    return m.astype(BF16)


def prep_in_maps(q, k, v, wq, wk, wv, wo):
    cmask = make_mask()
    qT = [np.ascontiguousarray(q[b].T).astype(BF16) for b in range(B)]
    kT = [np.ascontiguousarray(k[b].T).astype(BF16) for b in range(B)]
    vT = [np.ascontiguousarray(v[b].T).astype(BF16) for b in range(B)]
    in_maps = []
    for c in range(N_CORES):
        b, g = divmod(c, N_CORES // B)
        M = slice(g * M_CORE, (g + 1) * M_CORE)
        in_maps.append({
            "qT": qT[b], "kT": kT[b], "vT": vT[b],
            "wqT": np.ascontiguousarray(wq[M, :].T).astype(BF16),
            "wkT": np.ascontiguousarray(wk[M, :].T).astype(BF16),
            "wvT": np.ascontiguousarray(wv[M, :].T).astype(BF16),
            "woT": np.ascontiguousarray(wo[:, M].T).astype(BF16),
            "cmask": cmask,
        })
    return in_maps


def run(inputs, trace=False):
    nc = build_nc()
    in_maps = prep_in_maps(inputs["q"], inputs["k"], inputs["v"],
                           inputs["wq"], inputs["wk"], inputs["wv"], inputs["wo"])
    res = run_bass_kernel_spmd(nc, in_maps, core_ids=list(range(N_CORES)),
                               trace=trace)
    g = N_CORES // B
    out = np.stack([
        np.sum([res.results[b * g + i]["out"] for i in range(g)], axis=0)
        for b in range(B)
    ]).astype(np.float32)
    return out, res


def kernel(**inputs):
    out, _ = run(inputs, trace=False)
    return out


# revision 10
# speedup vs baseline: 1.0610x; 1.0565x over previous
"""Trainium2 Bass kernel for causal multi-head attention (B=2, S=2048, D=2048, H=16).

Sharding: DP over batch (2) x TP over heads (4 groups of 4 heads) = 8 cores.
Each core computes, for its batch b and head group g:
  - Q/K/V projections restricted to its 512 head-dims (transposed layouts)
  - causal attention for its 4 heads (scores produced directly transposed,
    softmax sums via ones-matmul, normalization folded in post-PV)
  - partial output projection through its 512 columns of wo
Host sums the 4 TP partials per batch (the "all-reduce" of the hint, done in
numpy on the gathered partials) and stacks the 2 batches.

All matmuls run in bf16 (fp32 PSUM accumulation); host pre-converts inputs.
"""

import numpy as np
import ml_dtypes

import concourse.bacc as bacc
import concourse.tile as tile
from concourse import mybir
from concourse.bass_utils import run_bass_kernel_spmd

BF16 = ml_dtypes.bfloat16

# Full problem sizes (hardcoded; grading calls kernel() with these shapes).
B, S, D, H = 2, 2048, 2048, 16
HD = 128          # head dim
P = 128           # SBUF partitions
CH = 512          # matmul moving-dim chunk
N_CORES = 8
M_CORE = D // 4   # head-dims per core (4 heads x 128)


def build_core_kernel(nc, io, S=S, D=D, M_CORE=M_CORE):
    """Emit the per-core Tile program (single causal-chunk pipeline).

    For each 512-wide sequence chunk c: project q/k/v for chunk c, run
    attention for all heads on queries of chunk c (keys/values from chunks
    <= c, which are already projected), then the wo partial for chunk c's
    rows. This keeps TensorE dense (projection matmuls of chunk c+1 fill the
    ACT-bound stretches of attention on chunk c).
    """
    E_T = D // P        # e (contraction) tiles
    S_T = S // P        # s tiles
    S_C = S // CH       # s chunks
    H_C = M_CORE // P   # heads on this core
    D_C = D // CH       # output e chunks
    SUB = CH // P       # 128-tiles per chunk
    bf = mybir.dt.bfloat16
    f32 = mybir.dt.float32
    SCALE = 1.0 / float(np.sqrt(np.float32(HD)))

    qT, kT, vT = io["qT"], io["kT"], io["vT"]
    wqT, wkT, wvT, woT = io["wqT"], io["wkT"], io["wvT"], io["woT"]
    cmask = io["cmask"]
    out = io["out"]

    import contextlib

    with tile.TileContext(nc) as tc, contextlib.ExitStack() as ctx:
        wpool = ctx.enter_context(tc.tile_pool(name="wpool", bufs=1))
        cons = ctx.enter_context(tc.tile_pool(name="cons", bufs=1))
        projo = ctx.enter_context(tc.tile_pool(name="projo", bufs=1))
        xin = ctx.enter_context(tc.tile_pool(name="xin", bufs=22))
        expp = ctx.enter_context(tc.tile_pool(name="expt", bufs=1))
        smp = ctx.enter_context(tc.tile_pool(name="sm", bufs=2))
        outp = ctx.enter_context(tc.tile_pool(name="outp", bufs=2))
        acc = ctx.enter_context(tc.tile_pool(name="ps_acc", bufs=3, space="PSUM"))
        pssc = ctx.enter_context(tc.tile_pool(name="ps_sc", bufs=2, space="PSUM"))
        pspv = ctx.enter_context(tc.tile_pool(name="ps_pv", bufs=2, space="PSUM"))
        psbc = ctx.enter_context(tc.tile_pool(name="ps_bc", bufs=1, space="PSUM"))

        # inputs arrive pre-tiled by the host: x: [E_T, S_C, P, CH] with each
        # [P, CH] tile a contiguous 128KB block (max DMA efficiency)
        x_tiled = {"q": qT, "k": kT, "v": vT}

        # projection weights resident (per-e-tile DMAs on the idle gpsimd
        # queue so they don't contend with the input stream)
        w_sbs = {}
        for name, w_dram in (("q", wqT), ("k", wkT), ("v", wvT)):
            w_sb = wpool.tile([P, E_T, M_CORE], bf, name=f"w{name}_sb")
            wt = w_dram.rearrange("(e p) m -> e p m", p=P)
            for e_t in range(E_T):
                nc.gpsimd.dma_start(out=w_sb[:, e_t, :], in_=wt[e_t])
            w_sbs[name] = w_sb

        mask_sb = cons.tile([P, SUB, CH], bf, name="mask_sb")
        nc.gpsimd.dma_start(out=mask_sb, in_=cmask.rearrange("p (s c) -> p s c", s=SUB))
        ones_mat = cons.tile([P, P], bf, name="ones_mat")
        nc.vector.memset(ones_mat, 1.0)
        woT_sb = cons.tile([P, H_C, D], bf, name="woT_sb")

        xqT_sb = projo.tile([P, H_C, S], bf, name="xqT_sb")
        xkT_sb = projo.tile([P, H_C, S], bf, name="xkT_sb")
        xv_sb = projo.tile([P, S_T, M_CORE], bf, name="xv_sb")
        attnT_sb = projo.tile([P, H_C, S], bf, name="attnT_sb")

        for c in range(S_C):
            csl = slice(c * CH, (c + 1) * CH)
            # ---- projections for chunk c ----
            for name in ("q", "k", "v"):
                w_sb = w_sbs[name]
                xts = []
                for e_t in range(E_T):
                    xt = xin.tile([P, CH], bf, tag="xin", name=f"x{name}{c}_{e_t}")
                    nc.sync.dma_start(out=xt, in_=x_tiled[name][e_t, c])
                    xts.append(xt)
                if name in ("q", "k"):
                    dst = xqT_sb if name == "q" else xkT_sb
                    for m in range(H_C):
                        ps = acc.tile([P, CH], f32, tag="acc", name=f"ps_{name}{c}_{m}")
                        for e_t in range(E_T):
                            nc.tensor.matmul(ps, lhsT=w_sb[:, e_t, m * P:(m + 1) * P],
                                             rhs=xts[e_t],
                                             start=(e_t == 0), stop=(e_t == E_T - 1))
                        nc.vector.tensor_copy(dst[:, m, csl], ps)
                else:
                    for s_sub in range(SUB):
                        ps = acc.tile([P, M_CORE], f32, tag="acc", name=f"ps_v{c}_{s_sub}")
                        for e_t in range(E_T):
                            nc.tensor.matmul(ps,
                                             lhsT=xts[e_t][:, s_sub * P:(s_sub + 1) * P],
                                             rhs=w_sb[:, e_t, :],
                                             start=(e_t == 0), stop=(e_t == E_T - 1))
                        nc.vector.tensor_copy(xv_sb[:, c * SUB + s_sub, :], ps)

            if c == 0:
                # wo weights: only needed from the first wo stage on; DMA here
                # so they don't delay the chunk-0 input stream
                for h in range(H_C):
                    nc.sync.dma_start(out=woT_sb[:, h, :],
                                      in_=woT[h * P:(h + 1) * P, :])

            # ---- attention for chunk c, all heads ----
            n_t = SUB * (c + 1)
            for h in range(H_C):
                exp_t = expp.tile([P, S_T, CH], bf, tag="expT", name=f"exp_{h}_{c}")
                for t in range(n_t):
                    ps_sc = pssc.tile([P, CH], f32, tag="sc", name=f"sc_{h}_{c}_{t}")
                    nc.tensor.matmul(ps_sc,
                                     lhsT=xkT_sb[:, h, t * P:(t + 1) * P],
                                     rhs=xqT_sb[:, h, csl],
                                     start=True, stop=True)
                    nc.scalar.activation(exp_t[:, t, :], ps_sc,
                                         mybir.ActivationFunctionType.Exp,
                                         scale=SCALE)
                    p = t - SUB * c
                    if p >= 0:
                        nc.vector.tensor_mul(exp_t[:, t, :], exp_t[:, t, :],
                                             mask_sb[:, p, :])
                # broadcast column-sums first: all-ones stationary -> every
                # psum partition gets sum_sk(exp); the slow DVE reciprocal
                # then overlaps the PV matmuls below
                ps_bc = psbc.tile([P, CH], f32, tag="bc", name=f"bc_{h}_{c}")
                for t in range(n_t):
                    nc.tensor.matmul(ps_bc, lhsT=ones_mat, rhs=exp_t[:, t, :],
                                     start=(t == 0), stop=(t == n_t - 1))
                bc_sb = smp.tile([P, CH], f32, tag="bcs", name=f"bcs_{h}_{c}")
                nc.vector.reciprocal(bc_sb, ps_bc)
                ps_pv = pspv.tile([P, CH], f32, tag="pv", name=f"pv_{h}_{c}")
                for t in range(n_t):
                    nc.tensor.matmul(ps_pv,
                                     lhsT=xv_sb[:, t, h * P:(h + 1) * P],
                                     rhs=exp_t[:, t, :],
                                     start=(t == 0), stop=(t == n_t - 1))
                nc.vector.tensor_mul(attnT_sb[:, h, csl], ps_pv, bc_sb)

            # ---- wo partial for chunk c's rows ----
            for s_t in range(c * SUB, (c + 1) * SUB):
                out_sb = outp.tile([P, D], f32, tag="out", name=f"out_{s_t}")
                for e_c in range(D_C):
                    ps = acc.tile([P, CH], f32, tag="acc", name=f"wo_{s_t}_{e_c}")
                    for h in range(H_C):
                        nc.tensor.matmul(ps,
                                         lhsT=attnT_sb[:, h, s_t * P:(s_t + 1) * P],
                                         rhs=woT_sb[:, h, e_c * CH:(e_c + 1) * CH],
                                         start=(h == 0), stop=(h == H_C - 1))
                    nc.vector.tensor_copy(out_sb[:, e_c * CH:(e_c + 1) * CH], ps)
                nc.sync.dma_start(out=out[s_t * P:(s_t + 1) * P, :], in_=out_sb)


def build_nc(S=S, D=D, M_CORE=M_CORE):
    nc = bacc.Bacc("TRN2", target_bir_lowering=False, debug=False, num_devices=N_CORES)
    bf = mybir.dt.bfloat16
    xshape = [D // P, S // CH, P, CH]
    io = {
        "qT": nc.dram_tensor("qT", xshape, bf, kind="ExternalInput").ap(),
        "kT": nc.dram_tensor("kT", xshape, bf, kind="ExternalInput").ap(),
        "vT": nc.dram_tensor("vT", xshape, bf, kind="ExternalInput").ap(),
        "wqT": nc.dram_tensor("wqT", [D, M_CORE], bf, kind="ExternalInput").ap(),
        "wkT": nc.dram_tensor("wkT", [D, M_CORE], bf, kind="ExternalInput").ap(),
        "wvT": nc.dram_tensor("wvT", [D, M_CORE], bf, kind="ExternalInput").ap(),
        "woT": nc.dram_tensor("woT", [M_CORE, D], bf, kind="ExternalInput").ap(),
        "cmask": nc.dram_tensor("cmask", [P, (CH // P) * CH], bf,
                                kind="ExternalInput").ap(),
        "out": nc.dram_tensor("out", [S, D], mybir.dt.float32,
                              kind="ExternalOutput").ap(),
    }
    build_core_kernel(nc, io, S=S, D=D, M_CORE=M_CORE)
    nc.compile()
    return nc


def make_mask():
    i = np.arange(P)[:, None]
    j = np.arange(CH)[None, :]
    m = np.concatenate(
        [(j >= P * p + i).astype(np.float32) for p in range(CH // P)], axis=1)
    return m.astype(BF16)


def tile_T(xT, D_=D, S_=S):
    """[D, S] bf16 -> tiled [D/P, S/CH, P, CH], each tile contiguous."""
    return np.ascontiguousarray(
        xT.reshape(D_ // P, P, S_ // CH, CH).transpose(0, 2, 1, 3))


def tile_xT(x):
    """[S, D] fp32 -> transposed+tiled [D/P, S/CH, P, CH] bf16."""
    return tile_T(x.T.astype(BF16))


def prep_in_maps(q, k, v, wq, wk, wv, wo):
    cmask = make_mask()
    qT = [tile_xT(q[b]) for b in range(B)]
    kT = [tile_xT(k[b]) for b in range(B)]
    vT = [tile_xT(v[b]) for b in range(B)]
    in_maps = []
    for c in range(N_CORES):
        b, g = divmod(c, N_CORES // B)
        M = slice(g * M_CORE, (g + 1) * M_CORE)
        in_maps.append({
            "qT": qT[b], "kT": kT[b], "vT": vT[b],
            "wqT": np.ascontiguousarray(wq[M, :].T).astype(BF16),
            "wkT": np.ascontiguousarray(wk[M, :].T).astype(BF16),
            "wvT": np.ascontiguousarray(wv[M, :].T).astype(BF16),
            "woT": np.ascontiguousarray(wo[:, M].T).astype(BF16),
            "cmask": cmask,
        })
    return in_maps


def run(inputs, trace=False):
    nc = build_nc()
    in_maps = prep_in_maps(inputs["q"], inputs["k"], inputs["v"],
                           inputs["wq"], inputs["wk"], inputs["wv"], inputs["wo"])
    res = run_bass_kernel_spmd(nc, in_maps, core_ids=list(range(N_CORES)),
                               trace=trace)
    g = N_CORES // B
    out = np.stack([
        np.sum([res.results[b * g + i]["out"] for i in range(g)], axis=0)
        for b in range(B)
    ]).astype(np.float32)
    return out, res


def kernel(**inputs):
    out, _ = run(inputs, trace=False)
    return out
